# revision 46
# baseline (speedup 1.0000x reference)
"""Trainium2 Bass kernel for nn_DigitCap (sparse_attention).

Math note: the reference's softmax is over a size-1 axis, so C == 1 exactly
and the whole N x N attention matrix A is dead code.  The computation
collapses to

    S[b,d,i]  = sum_{n,j} (1 + B[d,n]) * W[d,n,i,j] * U[b,n,j]
    out[b,d,:] = (1 - exp(-|S|)) * S / (|S| + 1e-7)

Sharding: split by digit capsule d (2 of 10 per core, zero-padded to a
uniform 2 so the SPMD program is identical on all 8 cores).

Perf structure (vs the fp32 baseline):
  * inputs stream as bf16 (tolerance is 2e-2; bf16 lands ~3e-3) -- halves
    DMA bytes and runs the PE at 1 cycle/row instead of fp32's 4.
  * every HWDGE DMA pays ~3us issue->semaphore latency regardless of
    size, so all inputs are packed into ONE dram tensor [bsc | W | U]
    and fetched with three big SP-ring DMAs.  The ACT engine issues no
    DMAs, so its table warm-up runs immediately and the 1.3us TDRAM
    table DMA clears the shared DMA device before the input transfers'
    descriptors are ready.
  * single PSUM bank accumulates all 32 chunks.  W is scaled by
    bscn = -(1+B) (so PSUM holds -S); the epilogue's (et-1)/norm factor
    is negated too and the signs cancel.
  * sqrt(x) = exp(0.5*ln(x)): Ln/Exp/Square all live in one ACT table
    set -- no table switch on the critical path.
  * epilogue: ACT squares S straight out of PSUM, DVE row-sums it; then
    Ln -> Exp(0.5) -> Exp(-1) on ACT while the DVE computes
    rec = 1/norm and oa = ps*rec under the final Exp; last op is one
    scalar_tensor_tensor: ot = (et - 1) * oa = S(1-exp(-|S|))/|S|.
"""

import numpy as np
from contextlib import ExitStack

import ml_dtypes

import concourse.bass as bass
import concourse.mybir as mybir
from concourse.bass_utils import run_bass_kernel_spmd

F32 = mybir.dt.float32
BF16 = mybir.dt.bfloat16
NPBF16 = ml_dtypes.bfloat16
AF = mybir.ActivationFunctionType
ALU = mybir.AluOpType
P = 128
D, DD, N, DP = 10, 16, 512, 8     # digit caps, digit dim, primary caps, primary dim
K = N * DP                         # 4096 contraction
NCHUNK = K // P                    # 32 chunks of 128 contraction rows
NCORES = 8
BFULL = 64
DC = 2                             # d's per core (8*2 = 16 slots >= 10 real)
DIC = DC * DD                      # 32 output cols per core
HC = NCHUNK // 2                   # 16 chunks per half
BCOL = NCHUNK * DC                 # bsc cols (64)
WCOL = NCHUNK * DIC                # w cols (1024)
UCOL = NCHUNK * BFULL              # u cols (2048)
W0 = BCOL                          # w offset in packed tensor
U0 = BCOL + WCOL                   # u offset in packed tensor
ALLCOL = BCOL + WCOL + UCOL        # 3136


def build_raw():
    nc = bass.Bass()
    in_t = nc.dram_tensor("in_t", [P, ALLCOL], BF16, kind="ExternalInput")
    out = nc.dram_tensor("out", [BFULL, DIC], F32, kind="ExternalOutput")

    UH = HC * BFULL                # u cols per half (1024)

    with ExitStack() as ctx:
        ab = ctx.enter_context(nc.sbuf_tensor("ab", [P, ALLCOL], BF16))
        ps = ctx.enter_context(nc.psum_tensor("ps", [BFULL, DIC], F32))
        sq = ctx.enter_context(nc.sbuf_tensor("sq", [BFULL, DIC], F32))
        ss = ctx.enter_context(nc.sbuf_tensor("ss", [BFULL, DC], F32))
        lt = ctx.enter_context(nc.sbuf_tensor("lt", [BFULL, DC], F32))
        normt = ctx.enter_context(nc.sbuf_tensor("norm", [BFULL, DC], F32))
        rec = ctx.enter_context(nc.sbuf_tensor("rec", [BFULL, DC], F32))
        et = ctx.enter_context(nc.sbuf_tensor("et", [BFULL, DC], F32))
        oa = ctx.enter_context(nc.sbuf_tensor("oa", [BFULL, DIC], F32))
        ot = ctx.enter_context(nc.sbuf_tensor("ot", [BFULL, DIC], F32))
        warm = ctx.enter_context(nc.sbuf_tensor("warm", [1, 4], F32))
        s_d1 = ctx.enter_context(nc.semaphore("s_d1"))
        s_u = [ctx.enter_context(nc.semaphore(f"s_u{h}")) for h in range(2)]
        s_wm = ctx.enter_context(nc.semaphore("s_wm"))
        s_dve = ctx.enter_context(nc.semaphore("s_dve"))
        s_pe = ctx.enter_context(nc.semaphore("s_pe"))
        s_hd = ctx.enter_context(nc.semaphore("s_hd"))
        s_a = ctx.enter_context(nc.semaphore("s_a"))
        s_nr = ctx.enter_context(nc.semaphore("s_nr"))
        s_e = ctx.enter_context(nc.semaphore("s_e"))
        s_v = ctx.enter_context(nc.semaphore("s_v"))
        s_fin = ctx.enter_context(nc.semaphore("s_fin"))
        s_out = ctx.enter_context(nc.semaphore("s_out"))

        bsc = ab[:, 0:BCOL]
        w_all = ab[:, W0:W0 + WCOL]
        u_all = ab[:, U0:U0 + UCOL]

        with nc.Block() as block:

            @block.sync
            def _(sync):
                # packed input: [bsc | W] first (gates scale -> PE start),
                # then the two U halves
                sync.dma_start(ab[:, 0:U0], in_t[:, 0:U0]).then_inc(s_d1, 16)
                for h in range(2):
                    sync.dma_start(
                        ab[:, U0 + h * UH:U0 + (h + 1) * UH],
                        in_t[:, U0 + h * UH:U0 + (h + 1) * UH],
                    ).then_inc(s_u[h], 16)
                sync.wait_ge(s_fin, 1)
                sync.dma_start(out[:, :], ot[:]).then_inc(s_out, 16)

            @block.scalar
            def _(scalar):
                # no DMAs on the ACT ring: warm the natural_log_exp table
                # set immediately so its TDRAM DMA clears the shared device
                # before the input transfers start
                scalar.wait_ge(s_wm, 1)
                scalar.activation(out=warm[:, 0:1], in_=warm[:, 1:2], func=AF.Ln)
                scalar.wait_ge(s_wm, 1)
                scalar.activation(out=warm[:, 2:3], in_=warm[:, 1:2], func=AF.Exp)
                # epilogue: sq = S^2 straight out of PSUM (DVE reduces it),
                # then norm = exp(0.5 ln ss) = sqrt(ss), et = exp(-norm)
                scalar.wait_ge(s_pe, 1)
                scalar.activation(
                    out=sq[:], in_=ps[:], func=AF.Square
                ).then_inc(s_hd, 1)
                scalar.wait_ge(s_hd, 2)
                scalar.activation(out=lt[:], in_=ss[:], func=AF.Ln).then_inc(s_a, 1)
                scalar.wait_ge(s_a, 1)
                scalar.activation(
                    out=normt[:], in_=lt[:], func=AF.Exp, scale=0.5
                ).then_inc(s_nr, 1)
                scalar.wait_ge(s_nr, 1)
                scalar.activation(
                    out=et[:], in_=normt[:], func=AF.Exp, scale=-1.0
                ).then_inc(s_e, 1)

            @block.vector
            def _(vector):
                # seed for the ACT table warm-up
                vector.memset(warm[:], 1.0).then_inc(s_wm, 1)
                # bscn = -(1 + B): the matmul then accumulates -S, and the
                # (et - 1)/norm factor is negated too, so the signs cancel
                vector.wait_ge(s_d1, 16)
                vector.tensor_scalar(
                    out=bsc, in0=bsc, scalar1=-1.0, scalar2=-1.0,
                    op0=ALU.mult, op1=ALU.add,
                ).then_inc(s_v, 1)
                # fused bscn * W in two halves so PE can start early
                vector.wait_ge(s_v, 1)
                for h in range(2):
                    w_v = w_all[:, h * HC * DIC:(h + 1) * HC * DIC].rearrange(
                        "p (c t i) -> p c t i", t=DC, i=DD
                    )
                    vector.tensor_mul(
                        out=w_v,
                        in0=bsc[:, h * HC * DC:(h + 1) * HC * DC]
                        .rearrange("p (c t) -> p c t", t=DC)
                        .broadcast_to([P, HC, DC, DD]),
                        in1=w_v,
                    ).then_inc(s_dve, 1)
                # epilogue head: one row-sum of the ACT-squared S
                vector.wait_ge(s_hd, 1)
                vector.tensor_reduce(
                    out=ss[:],
                    in_=sq[:].rearrange("b (t i) -> b t i", i=DD),
                    axis=mybir.AxisListType.X, op=ALU.add,
                ).then_inc(s_hd, 1)
                # tail: rec = 1/norm and oa = ps * rec = -S/norm both run
                # under the ACT et; final ot = (et - 1) * oa = S(1-et)/norm
                vector.wait_ge(s_nr, 1)
                vector.reciprocal(out=rec[:], in_=normt[:]).then_inc(s_v, 1)
                vector.wait_ge(s_v, 2)
                vector.tensor_mul(
                    out=oa[:].rearrange("b (t i) -> b t i", i=DD),
                    in0=ps[:].rearrange("b (t i) -> b t i", i=DD),
                    in1=rec[:].broadcast_to([BFULL, DC, DD]),
                ).then_inc(s_v, 1)
                vector.wait_ge(s_e, 1)
                vector.wait_ge(s_v, 3)
                vector.scalar_tensor_tensor(
                    out=ot[:].rearrange("b (t i) -> b t i", i=DD),
                    in0=et[:].broadcast_to([BFULL, DC, DD]),
                    scalar=1.0,
                    in1=oa[:].rearrange("b (t i) -> b t i", i=DD),
                    op0=ALU.subtract, op1=ALU.mult,
                ).then_inc(s_fin, 1)

            @block.tensor
            def _(tensor):
                for g in range(2):
                    tensor.wait_ge(s_dve, g + 1)
                    tensor.wait_ge(s_u[g], 16)
                    for k in range(HC):
                        c = g * HC + k
                        mm = tensor.matmul(
                            ps[:],
                            lhsT=u_all[:, c * BFULL:(c + 1) * BFULL],
                            rhs=w_all[:, c * DIC:(c + 1) * DIC],
                            start=(c == 0), stop=(c == NCHUNK - 1),
                            skip_group_check=True,
                        )
                mm.then_inc(s_pe, 1)

    return nc


_CACHE = {}


def _get_nc():
    if "nc" not in _CACHE:
        _CACHE["nc"] = build_raw()
    return _CACHE["nc"]


def prep_inputs(primary_caps, W, B):
    """Host-side layout prep + sharding (no arithmetic).

    Contraction row order: chunk c holds n in [c*16, (c+1)*16); within a
    chunk, partition p = j*16 + n_local.  Core c owns digit caps
    d in {2c, 2c+1} (zeros for the 6 pad slots on cores 5-7).
    Per-core packed input layout: [bsc (64) | W (1024) | U (2048)] cols.
    """
    U = np.asarray(primary_caps, dtype=np.float32)
    Wf = np.asarray(W, dtype=np.float32)
    Bf = np.asarray(B, dtype=np.float32).reshape(D, N)

    # U^T replicated: [p, (c b)]
    Unj = np.transpose(U, (1, 2, 0))  # n j b
    Ut = (
        Unj.reshape(NCHUNK, 16, DP, BFULL)
        .transpose(0, 2, 1, 3)
        .reshape(NCHUNK, P, BFULL)
        .transpose(1, 0, 2)
        .reshape(P, NCHUNK * BFULL)
    )

    # per-core W slice [p, (c, t, i)] and B slice [p, (c, t)]
    Wnj = np.transpose(Wf, (1, 3, 0, 2))  # n j d i
    Wc = (
        Wnj.reshape(NCHUNK, 16, DP, D, DD)
        .transpose(0, 2, 1, 3, 4)          # c j n_l d i
        .reshape(NCHUNK, P, D, DD)
        .transpose(1, 0, 2, 3)             # p c d i
    )
    Bn = Bf.reshape(D, NCHUNK, 16)         # d c n_l
    in_maps = []
    for core in range(NCORES):
        packed = np.zeros((P, ALLCOL), dtype=np.float32)
        packed[:, U0:] = Ut
        wt = packed[:, W0:U0].reshape(P, NCHUNK, DC, DD)
        bpt = np.zeros((16, NCHUNK, DC), dtype=np.float32)
        for t in range(DC):
            d = 2 * core + t
            if d < D:
                wt[:, :, t, :] = Wc[:, :, d, :]
                bpt[:, :, t] = Bn[d].T      # [n_l, c] -> ...
        packed[:, 0:BCOL] = np.broadcast_to(
            bpt.reshape(1, 16, BCOL), (DP, 16, BCOL)
        ).reshape(P, BCOL)
        in_maps.append({"in_t": packed.astype(NPBF16)})
    return in_maps


def kernel(primary_caps, W, B):
    nc = _get_nc()
    in_maps = prep_inputs(primary_caps, W, B)
    res = run_bass_kernel_spmd(nc, in_maps, core_ids=list(range(NCORES)))
    full = np.empty((BFULL, D, DD), dtype=np.float32)
    for core in range(NCORES):
        o = res.results[core]["out"].reshape(BFULL, DC, DD)
        for t in range(DC):
            d = 2 * core + t
            if d < D:
                full[:, d, :] = o[:, t, :]
    return full


# revision 48
# speedup vs baseline: 1.1365x; 1.1365x over previous
"""Trainium2 Bass kernel for nn_DigitCap (sparse_attention).

Math note: the reference's softmax is over a size-1 axis, so C == 1 exactly
and the whole N x N attention matrix A is dead code.  The computation
collapses to

    S[b,d,i]  = sum_{n,j} (1 + B[d,n]) * W[d,n,i,j] * U[b,n,j]
    out[b,d,:] = (1 - exp(-|S|)) * S / (|S| + 1e-7)

Sharding: split by digit capsule d (2 of 10 per core, zero-padded to a
uniform 2 so the SPMD program is identical on all 8 cores).

Perf structure (vs the fp32 baseline):
  * inputs stream as bf16 (tolerance is 2e-2; bf16 lands ~3e-3) -- halves
    DMA bytes and runs the PE at 1 cycle/row instead of fp32's 4.
  * every HWDGE DMA pays ~3us issue->semaphore latency regardless of
    size, so all inputs are packed into ONE dram tensor [bsc | W | U]
    and fetched with three big SP-ring DMAs.  The ACT engine issues no
    DMAs, so its table warm-up runs immediately and the 1.3us TDRAM
    table DMA clears the shared DMA device before the input transfers'
    descriptors are ready.
  * single PSUM bank accumulates all 32 chunks.  W is scaled by
    bscn = -(1+B) (so PSUM holds -S); the epilogue's (et-1)/norm factor
    is negated too and the signs cancel.
  * sqrt(x) = exp(0.5*ln(x)): Ln/Exp/Square all live in one ACT table
    set -- no table switch on the critical path.
  * epilogue: ACT squares S straight out of PSUM, DVE row-sums it; then
    Ln -> Exp(0.5) -> Exp(-1) on ACT while the DVE computes
    rec = 1/norm and oa = ps*rec under the final Exp; last op is one
    scalar_tensor_tensor: ot = (et - 1) * oa = S(1-exp(-|S|))/|S|.
"""

import numpy as np
from contextlib import ExitStack

import ml_dtypes

import concourse.bass as bass
import concourse.mybir as mybir
from concourse.bass_utils import run_bass_kernel_spmd

F32 = mybir.dt.float32
BF16 = mybir.dt.bfloat16
NPBF16 = ml_dtypes.bfloat16
AF = mybir.ActivationFunctionType
ALU = mybir.AluOpType
P = 128
D, DD, N, DP = 10, 16, 512, 8     # digit caps, digit dim, primary caps, primary dim
K = N * DP                         # 4096 contraction
NCHUNK = K // P                    # 32 chunks of 128 contraction rows
NCORES = 8
BFULL = 64
DC = 2                             # d's per core (8*2 = 16 slots >= 10 real)
DIC = DC * DD                      # 32 output cols per core
HC = NCHUNK // 2                   # 16 chunks per half
BCOL = NCHUNK * DC                 # bsc cols (64)
WCOL = NCHUNK * DIC                # w cols (1024)
UCOL = NCHUNK * BFULL              # u cols (2048)
W0 = BCOL                          # w offset in packed tensor
U0 = BCOL + WCOL                   # u offset in packed tensor
ALLCOL = BCOL + WCOL + UCOL        # 3136


def build_raw():
    nc = bass.Bass()
    in_t = nc.dram_tensor("in_t", [P, ALLCOL], BF16, kind="ExternalInput")
    out = nc.dram_tensor("out", [BFULL, DIC], F32, kind="ExternalOutput")

    UH = HC * BFULL                # u cols per half (1024)

    with ExitStack() as ctx:
        ab = ctx.enter_context(nc.sbuf_tensor("ab", [P, ALLCOL], BF16))
        ps = ctx.enter_context(nc.psum_tensor("ps", [BFULL, DIC], F32))
        sq = ctx.enter_context(nc.sbuf_tensor("sq", [BFULL, DIC], F32))
        ss = ctx.enter_context(nc.sbuf_tensor("ss", [BFULL, DC], F32))
        lt = ctx.enter_context(nc.sbuf_tensor("lt", [BFULL, DC], F32))
        normt = ctx.enter_context(nc.sbuf_tensor("norm", [BFULL, DC], F32))
        rec = ctx.enter_context(nc.sbuf_tensor("rec", [BFULL, DC], F32))
        et = ctx.enter_context(nc.sbuf_tensor("et", [BFULL, DC], F32))
        oa = ctx.enter_context(nc.sbuf_tensor("oa", [BFULL, DIC], F32))
        ot = ctx.enter_context(nc.sbuf_tensor("ot", [BFULL, DIC], F32))
        warm = ctx.enter_context(nc.sbuf_tensor("warm", [1, 4], F32))
        s_d1 = ctx.enter_context(nc.semaphore("s_d1"))
        s_u = [ctx.enter_context(nc.semaphore(f"s_u{h}")) for h in range(2)]
        s_wm = ctx.enter_context(nc.semaphore("s_wm"))
        s_dve = ctx.enter_context(nc.semaphore("s_dve"))
        s_pe = ctx.enter_context(nc.semaphore("s_pe"))
        s_hd = ctx.enter_context(nc.semaphore("s_hd"))
        s_a = ctx.enter_context(nc.semaphore("s_a"))
        s_nr = ctx.enter_context(nc.semaphore("s_nr"))
        s_e = ctx.enter_context(nc.semaphore("s_e"))
        s_v = ctx.enter_context(nc.semaphore("s_v"))
        s_fin = ctx.enter_context(nc.semaphore("s_fin"))
        s_out = ctx.enter_context(nc.semaphore("s_out"))

        bsc = ab[:, 0:BCOL]
        w_all = ab[:, W0:W0 + WCOL]
        u_all = ab[:, U0:U0 + UCOL]

        with nc.Block() as block:

            @block.sync
            def _(sync):
                # packed input: [bsc | W] first (gates scale -> PE start),
                # then the two U halves
                sync.dma_start(ab[:, 0:U0], in_t[:, 0:U0]).then_inc(s_d1, 16)
                for h in range(2):
                    sync.dma_start(
                        ab[:, U0 + h * UH:U0 + (h + 1) * UH],
                        in_t[:, U0 + h * UH:U0 + (h + 1) * UH],
                    ).then_inc(s_u[h], 16)
                sync.wait_ge(s_fin, 1)
                sync.dma_start(out[:, :], ot[:]).then_inc(s_out, 16)

            @block.scalar
            def _(scalar):
                # no DMAs on the ACT ring: warm the natural_log_exp table
                # set immediately so its TDRAM DMA clears the shared device
                # before the input transfers start
                scalar.wait_ge(s_wm, 1)
                scalar.activation(out=warm[:, 0:1], in_=warm[:, 1:2], func=AF.Ln)
                scalar.wait_ge(s_wm, 1)
                scalar.activation(out=warm[:, 2:3], in_=warm[:, 1:2], func=AF.Exp)
                # epilogue: sq = S^2 straight out of PSUM (DVE reduces it),
                # then norm = exp(0.5 ln ss) = sqrt(ss), et = exp(-norm)
                scalar.wait_ge(s_pe, 1)
                scalar.activation(
                    out=sq[:], in_=ps[:], func=AF.Square
                ).then_inc(s_hd, 1)
                scalar.wait_ge(s_hd, 2)
                scalar.activation(out=lt[:], in_=ss[:], func=AF.Ln).then_inc(s_a, 1)
                scalar.wait_ge(s_a, 1)
                scalar.activation(
                    out=normt[:], in_=lt[:], func=AF.Exp, scale=0.5
                ).then_inc(s_nr, 1)
                scalar.wait_ge(s_nr, 1)
                scalar.activation(
                    out=et[:], in_=normt[:], func=AF.Exp, scale=-1.0
                ).then_inc(s_e, 1)

            @block.vector
            def _(vector):
                # seed for the ACT table warm-up
                vector.memset(warm[:], 1.0).then_inc(s_wm, 1)
                # bscn = -(1 + B): the matmul then accumulates -S, and the
                # (et - 1)/norm factor is negated too, so the signs cancel
                vector.wait_ge(s_d1, 16)
                vector.tensor_scalar(
                    out=bsc, in0=bsc, scalar1=-1.0, scalar2=-1.0,
                    op0=ALU.mult, op1=ALU.add,
                ).then_inc(s_v, 1)
                # fused bscn * W in four quarters so PE can start after the
                # first 8 chunks are scaled; contiguous W as in0
                vector.wait_ge(s_v, 1)
                QD = NCHUNK // 4
                for q in range(4):
                    w_v = w_all[:, q * QD * DIC:(q + 1) * QD * DIC].rearrange(
                        "p (c t i) -> p c t i", t=DC, i=DD
                    )
                    vector.tensor_mul(
                        out=w_v,
                        in0=w_v,
                        in1=bsc[:, q * QD * DC:(q + 1) * QD * DC]
                        .rearrange("p (c t) -> p c t", t=DC)
                        .broadcast_to([P, QD, DC, DD]),
                    ).then_inc(s_dve, 1)
                # epilogue head: one row-sum of the ACT-squared S
                vector.wait_ge(s_hd, 1)
                vector.tensor_reduce(
                    out=ss[:],
                    in_=sq[:].rearrange("b (t i) -> b t i", i=DD),
                    axis=mybir.AxisListType.X, op=ALU.add,
                ).then_inc(s_hd, 1)
                # tail: rec = 1/norm and oa = ps * rec = -S/norm both run
                # under the ACT et; final ot = (et - 1) * oa = S(1-et)/norm
                vector.wait_ge(s_nr, 1)
                vector.reciprocal(out=rec[:], in_=normt[:]).then_inc(s_v, 1)
                vector.wait_ge(s_v, 2)
                vector.tensor_mul(
                    out=oa[:].rearrange("b (t i) -> b t i", i=DD),
                    in0=ps[:].rearrange("b (t i) -> b t i", i=DD),
                    in1=rec[:].broadcast_to([BFULL, DC, DD]),
                ).then_inc(s_v, 1)
                vector.wait_ge(s_e, 1)
                vector.wait_ge(s_v, 3)
                vector.scalar_tensor_tensor(
                    out=ot[:].rearrange("b (t i) -> b t i", i=DD),
                    in0=et[:].broadcast_to([BFULL, DC, DD]),
                    scalar=1.0,
                    in1=oa[:].rearrange("b (t i) -> b t i", i=DD),
                    op0=ALU.subtract, op1=ALU.mult,
                ).then_inc(s_fin, 1)

            @block.tensor
            def _(tensor):
                QD = NCHUNK // 4
                for g in range(4):
                    tensor.wait_ge(s_dve, g + 1)
                    if g % 2 == 0:
                        tensor.wait_ge(s_u[g // 2], 16)
                    for k in range(QD):
                        c = g * QD + k
                        mm = tensor.matmul(
                            ps[:],
                            lhsT=u_all[:, c * BFULL:(c + 1) * BFULL],
                            rhs=w_all[:, c * DIC:(c + 1) * DIC],
                            start=(c == 0), stop=(c == NCHUNK - 1),
                            skip_group_check=True,
                        )
                mm.then_inc(s_pe, 1)

    return nc


_CACHE = {}


def _get_nc():
    if "nc" not in _CACHE:
        _CACHE["nc"] = build_raw()
    return _CACHE["nc"]


def prep_inputs(primary_caps, W, B):
    """Host-side layout prep + sharding (no arithmetic).

    Contraction row order: chunk c holds n in [c*16, (c+1)*16); within a
    chunk, partition p = j*16 + n_local.  Core c owns digit caps
    d in {2c, 2c+1} (zeros for the 6 pad slots on cores 5-7).
    Per-core packed input layout: [bsc (64) | W (1024) | U (2048)] cols.
    """
    U = np.asarray(primary_caps, dtype=np.float32)
    Wf = np.asarray(W, dtype=np.float32)
    Bf = np.asarray(B, dtype=np.float32).reshape(D, N)

    # U^T replicated: [p, (c b)]
    Unj = np.transpose(U, (1, 2, 0))  # n j b
    Ut = (
        Unj.reshape(NCHUNK, 16, DP, BFULL)
        .transpose(0, 2, 1, 3)
        .reshape(NCHUNK, P, BFULL)
        .transpose(1, 0, 2)
        .reshape(P, NCHUNK * BFULL)
    )

    # per-core W slice [p, (c, t, i)] and B slice [p, (c, t)]
    Wnj = np.transpose(Wf, (1, 3, 0, 2))  # n j d i
    Wc = (
        Wnj.reshape(NCHUNK, 16, DP, D, DD)
        .transpose(0, 2, 1, 3, 4)          # c j n_l d i
        .reshape(NCHUNK, P, D, DD)
        .transpose(1, 0, 2, 3)             # p c d i
    )
    Bn = Bf.reshape(D, NCHUNK, 16)         # d c n_l
    in_maps = []
    for core in range(NCORES):
        packed = np.zeros((P, ALLCOL), dtype=np.float32)
        packed[:, U0:] = Ut
        wt = packed[:, W0:U0].reshape(P, NCHUNK, DC, DD)
        bpt = np.zeros((16, NCHUNK, DC), dtype=np.float32)
        for t in range(DC):
            d = 2 * core + t
            if d < D:
                wt[:, :, t, :] = Wc[:, :, d, :]
                bpt[:, :, t] = Bn[d].T      # [n_l, c] -> ...
        packed[:, 0:BCOL] = np.broadcast_to(
            bpt.reshape(1, 16, BCOL), (DP, 16, BCOL)
        ).reshape(P, BCOL)
        in_maps.append({"in_t": packed.astype(NPBF16)})
    return in_maps


def kernel(primary_caps, W, B):
    nc = _get_nc()
    in_maps = prep_inputs(primary_caps, W, B)
    res = run_bass_kernel_spmd(nc, in_maps, core_ids=list(range(NCORES)))
    full = np.empty((BFULL, D, DD), dtype=np.float32)
    for core in range(NCORES):
        o = res.results[core]["out"].reshape(BFULL, DC, DD)
        for t in range(DC):
            d = 2 * core + t
            if d < D:
                full[:, d, :] = o[:, t, :]
    return full
